# revision 33
# baseline (speedup 1.0000x reference)
"""ImprovedGRUCell Trainium2 kernel (8-core data-parallel over batch).

v6: 9-stage software pipeline, one super-tile (512 rows) per stage slot.

  Stage map for super-tile n (executed at iteration n+k):
    s0 @n   : DMA loads (j-half split), f32->bf16 casts (GPSIMD).
    s1 @n+1 : xT (kt-major) and hT (j-major blocks) via PE identity-matmul
              transposes, PSUM->SBUF copybacks on DVE.
    s2 @n+2 : z + attention matmuls (PE), half-size PSUM tiles.
    s3 @n+3 : tz = tanh(Sz/2), A = tanh(Sa), E = exp(va*A) fp8 (ACT).
    s4 @n+4 : att = E*hT fp8 (GPSIMD), softmax denominators (PE) + recip
              (DVE), zb = 0.5*tz+0.5 (DVE).
    s5 @n+5 : candidate pw (bf16) + pu (fp8 DoubleRow, K=256/shot) matmuls,
              sW = Copy(pw) on ACT (stt may read only one PSUM operand),
              Sh = sW + r*pu stt on DVE.
    s6 @n+6 : htl = tanh(Sh) (ACT).
    s7 @n+7 : blend h_t = h + zb*(htl - h) (DVE).
    s8 @n+8 : store.

  Every engine's per-iteration inputs are >= 1 iteration old (or produced
  early in the same iteration by design), so per-engine programs run nearly
  stall-free; PE work is emitted interleaved (cand / xpose / z / attn) to
  bridge the PSUM-ring WAR waits and keep the systolic array dense and at
  full p-state (keeping both transposes on PE beat a DMA-xbar variant by
  ~30% end-to-end in the timeline model: the xbar's latency + DMA-queue
  contention starved PE and reset its p-state).

  PSUM (8 banks): pz[128,512]x2 (2) + pa[128,512]x2 (2) + pst transpose
  staging [128,1024]bf16 (1) + pd (1) + pw[128,512] (1) + pu[128,512] (1).
  Half-size z/attention tiles double the ring slack so the PE matmuls of
  tile n+1 never wait on ACT's queue tail for tile n.

  Accuracy: bf16 everywhere except the attention softmax path (E, att,
  U_h-matmul operands in fp8e4; verified offline to keep rel-err at the
  bf16 level ~3e-3, budget 2e-2). Sigmoid avoided (table-set conflict with
  exp): z = 0.5*tanh(s/2)+0.5. ACT table set stays {Tanh, Exp, Copy}.
"""

import os
import sys

sys.path.insert(0, "/opt/trn_rl_repo")

import ml_dtypes
import numpy as np

import concourse.bass as bass
import concourse.mybir as mybir
from concourse import bacc, tile
from concourse.bass_utils import run_bass_kernel_spmd

B_TOTAL = 65536
N_CORES = 8
B_CORE = B_TOTAL // N_CORES  # 8192
D = 256
ST = 512  # batch rows per super-tile
N_ST = B_CORE // ST

F32 = mybir.dt.float32
BF16 = mybir.dt.bfloat16
FP8 = mybir.dt.float8e4
AF = mybir.ActivationFunctionType
ALU = mybir.AluOpType
DR = mybir.MatmulPerfMode.DoubleRow

_CACHE = {}

WNAMES = ("wzt", "uzt", "wat", "uat", "wht", "uht")


def build_nc(use_bias=False):
    nc = bacc.Bacc(
        "TRN2",
        target_bir_lowering=False,
        debug=False,
        enable_asserts=False,
        num_devices=N_CORES,
    )

    x_d = nc.dram_tensor("x", [B_CORE, D], F32, kind="ExternalInput")
    h_d = nc.dram_tensor("h", [B_CORE, D], F32, kind="ExternalInput")
    w_d = {
        n: nc.dram_tensor(
            n, [D, D], FP8 if n == "uht" else BF16, kind="ExternalInput"
        )
        for n in WNAMES
    }
    bz_d = nc.dram_tensor("bz", [1, D], BF16, kind="ExternalInput")
    bh_d = nc.dram_tensor("bh", [1, D], BF16, kind="ExternalInput")
    va_d = nc.dram_tensor("va", [D], F32, kind="ExternalInput")
    id_d = nc.dram_tensor("ident", [128, 128], BF16, kind="ExternalInput")
    out_d = nc.dram_tensor("out", [B_CORE, D], F32, kind="ExternalOutput")

    with tile.TileContext(nc) as tc:
        with (
            tc.tile_pool(name="wpool", bufs=1) as wp,
            tc.tile_pool(name="io", bufs=3) as io,
            tc.tile_pool(name="wk", bufs=3) as wk,
            tc.tile_pool(name="pz", bufs=2, space="PSUM") as pzp,
            tc.tile_pool(name="pa", bufs=2, space="PSUM") as pap,
            tc.tile_pool(name="pst", bufs=1, space="PSUM") as pst,
            tc.tile_pool(name="pd", bufs=1, space="PSUM") as pdp,
            tc.tile_pool(name="pw", bufs=1, space="PSUM") as pwp,
            tc.tile_pool(name="pu", bufs=1, space="PSUM") as pup,
        ):
            # ---- persistent weights -------------------------------------
            # w_sb[n]: [128 part = in-feature%128, (kt, hid)]; uht in fp8
            w_sb = {}
            for n in WNAMES:
                dt = FP8 if n == "uht" else BF16
                t = wp.tile([128, 2 * D], dt, tag=n)
                nc.sync.dma_start(
                    out=t.rearrange("p (kt h) -> p kt h", kt=2),
                    in_=w_d[n].ap().rearrange("(kt p) h -> p kt h", p=128),
                )
                w_sb[n] = t
            bz_sb = wp.tile([1, D], BF16, tag="bz")
            nc.sync.dma_start(out=bz_sb[:], in_=bz_d.ap())
            bh_sb = wp.tile([1, D], BF16, tag="bh")
            nc.sync.dma_start(out=bh_sb[:], in_=bh_d.ap())
            va_sb = wp.tile([128, 2], F32, tag="va")
            nc.sync.dma_start(
                out=va_sb[:], in_=va_d.ap().rearrange("(t p) -> p t", p=128)
            )
            ones_r = wp.tile([1, 128], BF16, tag="ones_r")  # K=1 lhsT for bias
            nc.vector.memset(ones_r[:], 1.0)
            ones_c = wp.tile([128, 1], BF16, tag="ones_c")  # rhs for denom
            nc.vector.memset(ones_c[:], 1.0)
            ident = wp.tile([128, 128], BF16, tag="ident")
            nc.sync.dma_start(out=ident[:], in_=id_d.ap())

            uht3 = w_sb["uht"].rearrange("p (t h) -> p t h", t=2)
            # KREP>1 repeats the whole batch loop for wall-clock benching.
            KREP = int(os.environ.get("KREP", "1"))
            S = [dict() for _ in range(N_ST)]

            def s0(i):
                """Loads + casts + h xbar transpose."""
                s = S[i]
                b0 = i * ST
                s["xn"] = xn = io.tile(
                    [128, 4 * D], F32, tag="xn", name="xn", bufs=4
                )
                s["hn"] = hn = io.tile(
                    [128, 4 * D], F32, tag="hn", name="hn", bufs=9
                )
                s["xb"] = xb = wk.tile(
                    [128, 4 * D], BF16, tag="xb", name="xb", bufs=4
                )
                s["hb"] = hb = wk.tile(
                    [128, 4 * D], BF16, tag="hb", name="hb", bufs=9
                )
                for src, dst, cst in ((x_d, xn, xb), (h_d, hn, hb)):
                    for jh in range(2):
                        r0 = b0 + jh * 256
                        nc.sync.dma_start(
                            out=dst.rearrange("p (j k) -> p j k", j=4)[
                                :, jh * 2 : jh * 2 + 2
                            ],
                            in_=src.ap()[r0 : r0 + 256, :].rearrange(
                                "(j p) k -> p j k", p=128
                            ),
                        )
                        nc.gpsimd.tensor_copy(
                            cst[:, jh * 512 : (jh + 1) * 512],
                            dst[:, jh * 512 : (jh + 1) * 512],
                        )

            def s1(i):
                """xT via PE transpose (kt-major: xT[kk, kt*512+j*128+b]);
                hT likewise on PE but stored j-major (hT[kk,(j*2+kt)*128+b])
                so downstream slicing matches the xbar layout."""
                s = S[i]
                xb, hb = s["xb"], s["hb"]
                s["xT"] = xT = wk.tile(
                    [128, 4 * D], BF16, tag="xT", name="xT", bufs=5
                )
                s["hT"] = hT = wk.tile(
                    [128, 4 * D], BF16, tag="hT", name="hT", bufs=5
                )
                ptx = pst.tile([128, 4 * D], BF16, tag="pst")
                ptx3 = ptx.rearrange("k (kt j b) -> k (kt j) b", kt=2, j=4)
                for kt in range(2):
                    for j in range(4):
                        nc.tensor.transpose(
                            ptx3[:, kt * 4 + j],
                            xb[:, j * D + kt * 128 : j * D + (kt + 1) * 128],
                            ident[:],
                        )
                nc.vector.tensor_copy(xT[:], ptx[:])
                pth = pst.tile([128, 4 * D], BF16, tag="pst")
                pth3 = pth.rearrange("k (j kt b) -> k (j kt) b", kt=2, j=4)
                for kt in range(2):
                    for j in range(4):
                        nc.tensor.transpose(
                            pth3[:, j * 2 + kt],
                            hb[:, j * D + kt * 128 : j * D + (kt + 1) * 128],
                            ident[:],
                        )
                nc.vector.tensor_copy(hT[:], pth[:])

            def s2_zmm(i, jp):
                """z-branch matmuls for j-pair jp into a half-size tile."""
                s = S[i]
                xT, hT = s["xT"], s["hT"]
                pz = pzp.tile([128, 2 * D], F32, tag="pz", name="pz")
                s[f"pz{jp}"] = pz
                for jj in range(2):
                    j = jp * 2 + jj
                    sl = slice(jj * D, (jj + 1) * D)
                    for kt in range(2):
                        nc.tensor.matmul(
                            pz[:, sl],
                            xT[:, kt * 512 + j * 128 : kt * 512 + (j + 1) * 128],
                            w_sb["wzt"][:, kt * D : (kt + 1) * D],
                            start=(kt == 0),
                            stop=False,
                        )
                    for kt in range(2):
                        nc.tensor.matmul(
                            pz[:, sl],
                            hT[:, (j * 2 + kt) * 128 : (j * 2 + kt + 1) * 128],
                            w_sb["uzt"][:, kt * D : (kt + 1) * D],
                            start=False,
                            stop=(not use_bias and kt == 1),
                        )
                    if use_bias:
                        nc.tensor.matmul(
                            pz[:, sl], ones_r[:], bz_sb[:], start=False, stop=True
                        )

            def s2_amm(i, ht):
                """Attention matmuls for hidden-half ht (own half tile)."""
                s = S[i]
                xT, hT = s["xT"], s["hT"]
                pa = pap.tile([128, 2 * D], F32, tag="pa", name="pa")
                s[f"pa{ht}"] = pa
                hT4 = hT.rearrange("k (j kt b) -> k j kt b", j=4, kt=2)
                for kt in range(2):
                    nc.tensor.matmul(
                        pa[:],
                        w_sb["wat"][
                            :, kt * D + ht * 128 : kt * D + ht * 128 + 128
                        ],
                        xT[:, kt * 512 : (kt + 1) * 512],
                        start=(kt == 0),
                        stop=False,
                    )
                for kt in range(2):
                    nc.tensor.matmul(
                        pa[:],
                        w_sb["uat"][
                            :, kt * D + ht * 128 : kt * D + ht * 128 + 128
                        ],
                        hT4[:, :, kt, :],
                        start=False,
                        stop=(kt == 1),
                    )

            def s3_tz(i, jp):
                s = S[i]
                if "tz" not in s:
                    s["tz"] = wk.tile(
                        [128, 4 * D], BF16, tag="tz", name="tz", bufs=3
                    )
                tz = s["tz"]
                nc.scalar.activation(
                    tz[:, jp * 512 : (jp + 1) * 512],
                    s[f"pz{jp}"][:],
                    AF.Tanh,
                    scale=0.5,
                )

            def s3_ae(i, ht):
                s = S[i]
                if "A" not in s:
                    s["A"] = wk.tile(
                        [128, 4 * D], BF16, tag="A", name="A_sb", bufs=3
                    )
                    s["E"] = wk.tile(
                        [128, 4 * D], FP8, tag="E", name="E_sb", bufs=3
                    )
                A_sb, E_sb = s["A"], s["E"]
                sl = slice(ht * 512, (ht + 1) * 512)
                nc.scalar.activation(A_sb[:, sl], s[f"pa{ht}"][:], AF.Tanh)
                nc.scalar.activation(
                    E_sb[:, sl], A_sb[:, sl], AF.Exp, scale=va_sb[:, ht : ht + 1]
                )

            def s4(i):
                """att product (Pool), denominators (PE) + recip, zb."""
                s = S[i]
                E_sb, hT = s["E"], s["hT"]
                # att[k, ht*512 + j*128 + b] = E * hT (hT blocks j-major)
                s["att"] = att = wk.tile(
                    [128, 4 * D], FP8, tag="att", name="att", bufs=3
                )
                att5 = att.rearrange("k (t j b) -> k t j b", t=2, j=4)
                E5 = E_sb.rearrange("k (t j b) -> k t j b", t=2, j=4)
                hT4 = hT.rearrange("k (j kt b) -> k j kt b", j=4, kt=2)
                for ht in range(2):
                    nc.gpsimd.tensor_mul(
                        att5[:, ht], E5[:, ht], hT4[:, :, ht, :]
                    )
                pd = pdp.tile([128, 4], F32, tag="pd")
                for j in range(4):
                    for ht in range(2):
                        nc.tensor.matmul(
                            pd[:, j : j + 1],
                            E_sb[:, ht * 512 + j * 128 : ht * 512 + (j + 1) * 128],
                            ones_c[:],
                            start=(ht == 0),
                            stop=(ht == 1),
                        )
                s["r"] = r_sb = wk.tile(
                    [128, 4], F32, tag="r", name="r_sb", bufs=3
                )
                nc.vector.reciprocal(r_sb[:], pd[:])
                s["zb"] = zb = wk.tile(
                    [128, 4 * D], BF16, tag="zb", name="zb", bufs=5
                )
                nc.vector.tensor_scalar(
                    zb[:], s["tz"][:], 0.5, 0.5, op0=ALU.mult, op1=ALU.add
                )

            def s5_mm(i, jp):
                """Candidate matmuls for j-pair jp."""
                s = S[i]
                xT, att = s["xT"], s["att"]
                att4 = att.rearrange("k (t j b) -> k t j b", t=2, j=4)
                pw = pwp.tile([128, 2 * D], F32, tag="pw")
                s[f"pw{jp}"] = pw
                for jj in range(2):
                    j = jp * 2 + jj
                    for kt in range(2):
                        nc.tensor.matmul(
                            pw[:, jj * D : (jj + 1) * D],
                            xT[:, kt * 512 + j * 128 : kt * 512 + (j + 1) * 128],
                            w_sb["wht"][:, kt * D : (kt + 1) * D],
                            start=(kt == 0),
                            stop=(not use_bias and kt == 1),
                        )
                    if use_bias:
                        nc.tensor.matmul(
                            pw[:, jj * D : (jj + 1) * D],
                            ones_r[:],
                            bh_sb[:],
                            start=False,
                            stop=True,
                        )
                pu = pup.tile([128, 2 * D], F32, tag="pu")
                s[f"pu{jp}"] = pu
                for jj in range(2):
                    j = jp * 2 + jj
                    # attended @ U_h^T: one fp8 DoubleRow matmul (K=256)
                    nc.tensor.matmul(
                        pu[:, jj * D : (jj + 1) * D],
                        att4[:, :, j, :],
                        uht3[:],
                        start=True,
                        stop=True,
                        perf_mode=DR,
                    )

            def s5_fix(i, jp):
                """sW copy (ACT) + stt (DVE) for j-pair jp."""
                s = S[i]
                if "Sh" not in s:
                    s["Sh"] = wk.tile(
                        [128, 4 * D], F32, tag="Sh", name="Sh", bufs=3
                    )
                    s["sW"] = wk.tile(
                        [128, 4 * D], BF16, tag="sW", name="sW", bufs=3
                    )
                Sh, sW = s["Sh"], s["sW"]
                pw, pu = s[f"pw{jp}"], s[f"pu{jp}"]
                nc.scalar.activation(
                    sW[:, jp * 512 : (jp + 1) * 512], pw[:], AF.Copy
                )
                for jj in range(2):
                    j = jp * 2 + jj
                    nc.vector.scalar_tensor_tensor(
                        Sh[:, j * D : (j + 1) * D],
                        pu[:, jj * D : (jj + 1) * D],
                        s["r"][:, j : j + 1],
                        sW[:, j * D : (j + 1) * D],
                        op0=ALU.mult,
                        op1=ALU.add,
                    )

            def s6(i):
                s = S[i]
                s["htl"] = htl = wk.tile(
                    [128, 4 * D], BF16, tag="htl", name="htl", bufs=3
                )
                nc.scalar.activation(htl[:], s["Sh"][:], AF.Tanh)

            def s7(i):
                """Blend: h_t = h + zb*(h~ - h)."""
                s = S[i]
                s_bl = wk.tile(
                    [128, 4 * D], BF16, tag="s_bl", name="s_bl", bufs=3
                )
                nc.vector.tensor_sub(s_bl[:], s["htl"][:], s["hb"][:])
                m_bl = wk.tile(
                    [128, 4 * D], BF16, tag="m_bl", name="m_bl", bufs=3
                )
                nc.vector.tensor_mul(m_bl[:], s["zb"][:], s_bl[:])
                s["ot"] = ot = io.tile(
                    [128, 4 * D], F32, tag="ot", name="ot", bufs=4
                )
                nc.vector.tensor_add(ot[:], m_bl[:], s["hn"][:])

            def s8(i):
                s = S[i]
                b0 = i * ST
                nc.sync.dma_start(
                    out=out_d.ap()[b0 : b0 + ST, :].rearrange(
                        "(j p) k -> p j k", p=128
                    ),
                    in_=s["ot"].rearrange("p (j k) -> p j k", j=4),
                )
                S[i] = {}

            def alive(j):
                return 0 <= j < N_ST

            for _rep in range(KREP):
                for k in range(N_ST + 8):
                    # ACT leads with tz (unblocks PE z via the pz ring).
                    if alive(k - 3):
                        s3_tz(k - 3, 0)
                        s3_tz(k - 3, 1)
                    if alive(k - 7):
                        s7(k - 7)
                    # PE block, interleaved to bridge ring WARs:
                    if alive(k - 5):
                        s5_mm(k - 5, 0)
                    if alive(k - 1):
                        s1(k - 1)
                    if alive(k - 2):
                        s2_zmm(k - 2, 0)
                    if alive(k - 5):
                        s5_fix(k - 5, 0)
                        s5_mm(k - 5, 1)
                    if alive(k - 2):
                        s2_zmm(k - 2, 1)
                    if alive(k - 5):
                        s5_fix(k - 5, 1)
                    if alive(k - 4):
                        s4(k - 4)
                    if alive(k - 2):
                        s2_amm(k - 2, 0)
                    if alive(k - 3):
                        s3_ae(k - 3, 0)
                    if alive(k - 2):
                        s2_amm(k - 2, 1)
                    if alive(k - 3):
                        s3_ae(k - 3, 1)
                    if alive(k - 6):
                        s6(k - 6)
                    if alive(k):
                        s0(k)
                    if alive(k - 8):
                        s8(k - 8)

    nc.compile()
    return nc


LAST_RESULTS = None


def kernel(x, h_prev, W_z, U_z, b_z, W_a, U_a, v_a, W_h, U_h, b_h):
    global LAST_RESULTS
    use_bias = bool(np.any(np.asarray(b_z)) or np.any(np.asarray(b_h)))
    key = ("nc", use_bias)
    if key not in _CACHE:
        _CACHE[key] = build_nc(use_bias)
    nc = _CACHE[key]

    bf = ml_dtypes.bfloat16
    x = np.ascontiguousarray(np.asarray(x, dtype=np.float32))
    h_prev = np.ascontiguousarray(np.asarray(h_prev, dtype=np.float32))
    wmats = {
        "wzt": W_z,
        "uzt": U_z,
        "wat": W_a,
        "uat": U_a,
        "wht": W_h,
        "uht": U_h,
    }
    f8 = ml_dtypes.float8_e4m3fn
    common = {
        n: np.ascontiguousarray(
            np.asarray(m, dtype=np.float32).T.astype(f8 if n == "uht" else bf)
        )
        for n, m in wmats.items()
    }
    common["bz"] = np.asarray(b_z, dtype=np.float32).reshape(1, D).astype(bf)
    common["bh"] = np.asarray(b_h, dtype=np.float32).reshape(1, D).astype(bf)
    common["va"] = np.ascontiguousarray(np.asarray(v_a, dtype=np.float32))
    common["ident"] = np.eye(128, dtype=bf)

    in_maps = []
    for c in range(N_CORES):
        m = dict(common)
        m["x"] = x[c * B_CORE : (c + 1) * B_CORE]
        m["h"] = h_prev[c * B_CORE : (c + 1) * B_CORE]
        in_maps.append(m)

    LAST_RESULTS = run_bass_kernel_spmd(nc, in_maps, core_ids=list(range(N_CORES)))
    outs = LAST_RESULTS.results
    return np.concatenate([outs[c]["out"] for c in range(N_CORES)], axis=0)


# revision 36
# speedup vs baseline: 1.0335x; 1.0335x over previous
"""ImprovedGRUCell Trainium2 kernel (8-core data-parallel over batch).

v6: 9-stage software pipeline, one super-tile (512 rows) per stage slot.

  Stage map for super-tile n (executed at iteration n+k):
    s0 @n   : DMA loads (j-half split), f32->bf16 casts (GPSIMD).
    s1 @n+1 : xT (kt-major) and hT (j-major blocks) via PE identity-matmul
              transposes, PSUM->SBUF copybacks on DVE.
    s2 @n+2 : z + attention matmuls (PE), half-size PSUM tiles.
    s3 @n+3 : tz = tanh(Sz/2), A = tanh(Sa), E = exp(va*A) fp8 (ACT).
    s4 @n+4 : att = E*hT fp8 (GPSIMD), softmax denominators (PE) + recip
              (DVE), zb = 0.5*tz+0.5 (DVE).
    s5 @n+5 : candidate pw (bf16) + pu (fp8 DoubleRow, K=256/shot) matmuls,
              sW = Copy(pw) on ACT (stt may read only one PSUM operand),
              Sh = sW + r*pu stt on DVE.
    s6 @n+6 : htl = tanh(Sh) (ACT).
    s7 @n+7 : blend h_t = h + zb*(htl - h) (DVE).
    s8 @n+8 : store.

  Every engine's per-iteration inputs are >= 1 iteration old (or produced
  early in the same iteration by design), so per-engine programs run nearly
  stall-free; PE work is emitted interleaved (cand / xpose / z / attn) to
  bridge the PSUM-ring WAR waits and keep the systolic array dense and at
  full p-state (keeping both transposes on PE beat a DMA-xbar variant by
  ~30% end-to-end in the timeline model: the xbar's latency + DMA-queue
  contention starved PE and reset its p-state).

  PSUM (8 banks): pz[128,512]x2 (2) + pa[128,512]x2 (2) + pst transpose
  staging [128,1024]bf16 (1) + pd (1) + pw[128,512] (1) + pu[128,512] (1).
  Half-size z/attention tiles double the ring slack so the PE matmuls of
  tile n+1 never wait on ACT's queue tail for tile n.

  Accuracy: bf16 everywhere except the attention softmax path (E, att,
  U_h-matmul operands in fp8e4; verified offline to keep rel-err at the
  bf16 level ~3e-3, budget 2e-2). Sigmoid avoided (table-set conflict with
  exp): z = 0.5*tanh(s/2)+0.5. ACT table set stays {Tanh, Exp, Copy}.
"""

import os
import sys

sys.path.insert(0, "/opt/trn_rl_repo")

import ml_dtypes
import numpy as np

import concourse.bass as bass
import concourse.mybir as mybir
from concourse import bacc, tile
from concourse.bass_utils import run_bass_kernel_spmd

B_TOTAL = 65536
N_CORES = 8
B_CORE = B_TOTAL // N_CORES  # 8192
D = 256
ST = 512  # batch rows per super-tile
N_ST = B_CORE // ST

F32 = mybir.dt.float32
BF16 = mybir.dt.bfloat16
FP8 = mybir.dt.float8e4
AF = mybir.ActivationFunctionType
ALU = mybir.AluOpType
DR = mybir.MatmulPerfMode.DoubleRow

_CACHE = {}

WNAMES = ("wzt", "uzt", "wat", "uat", "wht", "uht")


def build_nc(use_bias=False):
    nc = bacc.Bacc(
        "TRN2",
        target_bir_lowering=False,
        debug=False,
        enable_asserts=False,
        num_devices=N_CORES,
    )

    x_d = nc.dram_tensor("x", [B_CORE, D], F32, kind="ExternalInput")
    h_d = nc.dram_tensor("h", [B_CORE, D], F32, kind="ExternalInput")
    w_d = {
        n: nc.dram_tensor(
            n, [D, D], FP8 if n == "uht" else BF16, kind="ExternalInput"
        )
        for n in WNAMES
    }
    bz_d = nc.dram_tensor("bz", [1, D], BF16, kind="ExternalInput")
    bh_d = nc.dram_tensor("bh", [1, D], BF16, kind="ExternalInput")
    va_d = nc.dram_tensor("va", [D], F32, kind="ExternalInput")
    id_d = nc.dram_tensor("ident", [128, 128], BF16, kind="ExternalInput")
    out_d = nc.dram_tensor("out", [B_CORE, D], F32, kind="ExternalOutput")

    with tile.TileContext(nc) as tc:
        with (
            tc.tile_pool(name="wpool", bufs=1) as wp,
            tc.tile_pool(name="io", bufs=3) as io,
            tc.tile_pool(name="wk", bufs=3) as wk,
            tc.tile_pool(name="pz", bufs=2, space="PSUM") as pzp,
            tc.tile_pool(name="pa", bufs=2, space="PSUM") as pap,
            tc.tile_pool(name="pst", bufs=1, space="PSUM") as pst,
            tc.tile_pool(name="pd", bufs=1, space="PSUM") as pdp,
            tc.tile_pool(name="pw", bufs=1, space="PSUM") as pwp,
            tc.tile_pool(name="pu", bufs=1, space="PSUM") as pup,
        ):
            # ---- persistent weights -------------------------------------
            # w_sb[n]: [128 part = in-feature%128, (kt, hid)]; uht in fp8.
            # Tiles are allocated here but the weight DMAs are EMITTED after
            # tile 0's x/h loads (load_weights below): the DMA queue is
            # serial and the first super-tile's data is needed first, while
            # weights aren't consumed until the first matmuls two
            # iterations later.
            w_sb = {}
            for n in WNAMES:
                dt = FP8 if n == "uht" else BF16
                w_sb[n] = wp.tile([128, 2 * D], dt, tag=n, name=n)
            bz_sb = wp.tile([1, D], BF16, tag="bz")
            bh_sb = wp.tile([1, D], BF16, tag="bh")
            va_sb = wp.tile([128, 2], F32, tag="va")
            ones_r = wp.tile([1, 128], BF16, tag="ones_r")  # K=1 lhsT for bias
            nc.vector.memset(ones_r[:], 1.0)
            ones_c = wp.tile([128, 1], BF16, tag="ones_c")  # rhs for denom
            nc.vector.memset(ones_c[:], 1.0)
            ident = wp.tile([128, 128], BF16, tag="ident")

            def load_weights():
                nc.sync.dma_start(out=ident[:], in_=id_d.ap())
                for n in WNAMES:
                    nc.sync.dma_start(
                        out=w_sb[n].rearrange("p (kt h) -> p kt h", kt=2),
                        in_=w_d[n].ap().rearrange("(kt p) h -> p kt h", p=128),
                    )
                nc.sync.dma_start(out=bz_sb[:], in_=bz_d.ap())
                nc.sync.dma_start(out=bh_sb[:], in_=bh_d.ap())
                nc.sync.dma_start(
                    out=va_sb[:], in_=va_d.ap().rearrange("(t p) -> p t", p=128)
                )

            uht3 = w_sb["uht"].rearrange("p (t h) -> p t h", t=2)
            # KREP>1 repeats the whole batch loop for wall-clock benching.
            KREP = int(os.environ.get("KREP", "1"))
            S = [dict() for _ in range(N_ST)]

            def s0(i):
                """Loads + casts + h xbar transpose."""
                s = S[i]
                b0 = i * ST
                s["xn"] = xn = io.tile(
                    [128, 4 * D], F32, tag="xn", name="xn", bufs=4
                )
                s["hn"] = hn = io.tile(
                    [128, 4 * D], F32, tag="hn", name="hn", bufs=9
                )
                s["xb"] = xb = wk.tile(
                    [128, 4 * D], BF16, tag="xb", name="xb", bufs=4
                )
                s["hb"] = hb = wk.tile(
                    [128, 4 * D], BF16, tag="hb", name="hb", bufs=9
                )
                for src, dst, cst in ((x_d, xn, xb), (h_d, hn, hb)):
                    nc.sync.dma_start(
                        out=dst.rearrange("p (j k) -> p j k", j=4),
                        in_=src.ap()[b0 : b0 + ST, :].rearrange(
                            "(j p) k -> p j k", p=128
                        ),
                    )
                    nc.gpsimd.tensor_copy(cst[:], dst[:])

            def s1(i):
                """xT via PE transpose (kt-major: xT[kk, kt*512+j*128+b]);
                hT likewise on PE but stored j-major (hT[kk,(j*2+kt)*128+b])
                so downstream slicing matches the xbar layout."""
                s = S[i]
                xb, hb = s["xb"], s["hb"]
                s["xT"] = xT = wk.tile(
                    [128, 4 * D], BF16, tag="xT", name="xT", bufs=5
                )
                s["hT"] = hT = wk.tile(
                    [128, 4 * D], BF16, tag="hT", name="hT", bufs=5
                )
                ptx = pst.tile([128, 4 * D], BF16, tag="pst")
                ptx3 = ptx.rearrange("k (kt j b) -> k (kt j) b", kt=2, j=4)
                for kt in range(2):
                    for j in range(4):
                        nc.tensor.transpose(
                            ptx3[:, kt * 4 + j],
                            xb[:, j * D + kt * 128 : j * D + (kt + 1) * 128],
                            ident[:],
                        )
                nc.vector.tensor_copy(xT[:], ptx[:])
                pth = pst.tile([128, 4 * D], BF16, tag="pst")
                pth3 = pth.rearrange("k (j kt b) -> k (j kt) b", kt=2, j=4)
                for kt in range(2):
                    for j in range(4):
                        nc.tensor.transpose(
                            pth3[:, j * 2 + kt],
                            hb[:, j * D + kt * 128 : j * D + (kt + 1) * 128],
                            ident[:],
                        )
                nc.vector.tensor_copy(hT[:], pth[:])

            def s2_zmm(i, jp):
                """z-branch matmuls for j-pair jp into a half-size tile."""
                s = S[i]
                xT, hT = s["xT"], s["hT"]
                pz = pzp.tile([128, 2 * D], F32, tag="pz", name="pz")
                s[f"pz{jp}"] = pz
                for jj in range(2):
                    j = jp * 2 + jj
                    sl = slice(jj * D, (jj + 1) * D)
                    for kt in range(2):
                        nc.tensor.matmul(
                            pz[:, sl],
                            xT[:, kt * 512 + j * 128 : kt * 512 + (j + 1) * 128],
                            w_sb["wzt"][:, kt * D : (kt + 1) * D],
                            start=(kt == 0),
                            stop=False,
                        )
                    for kt in range(2):
                        nc.tensor.matmul(
                            pz[:, sl],
                            hT[:, (j * 2 + kt) * 128 : (j * 2 + kt + 1) * 128],
                            w_sb["uzt"][:, kt * D : (kt + 1) * D],
                            start=False,
                            stop=(not use_bias and kt == 1),
                        )
                    if use_bias:
                        nc.tensor.matmul(
                            pz[:, sl], ones_r[:], bz_sb[:], start=False, stop=True
                        )

            def s2_amm(i, ht):
                """Attention matmuls for hidden-half ht (own half tile)."""
                s = S[i]
                xT, hT = s["xT"], s["hT"]
                pa = pap.tile([128, 2 * D], F32, tag="pa", name="pa")
                s[f"pa{ht}"] = pa
                hT4 = hT.rearrange("k (j kt b) -> k j kt b", j=4, kt=2)
                for kt in range(2):
                    nc.tensor.matmul(
                        pa[:],
                        w_sb["wat"][
                            :, kt * D + ht * 128 : kt * D + ht * 128 + 128
                        ],
                        xT[:, kt * 512 : (kt + 1) * 512],
                        start=(kt == 0),
                        stop=False,
                    )
                for kt in range(2):
                    nc.tensor.matmul(
                        pa[:],
                        w_sb["uat"][
                            :, kt * D + ht * 128 : kt * D + ht * 128 + 128
                        ],
                        hT4[:, :, kt, :],
                        start=False,
                        stop=(kt == 1),
                    )

            def s3_tz(i, jp):
                s = S[i]
                if "tz" not in s:
                    s["tz"] = wk.tile(
                        [128, 4 * D], BF16, tag="tz", name="tz", bufs=3
                    )
                tz = s["tz"]
                nc.scalar.activation(
                    tz[:, jp * 512 : (jp + 1) * 512],
                    s[f"pz{jp}"][:],
                    AF.Tanh,
                    scale=0.5,
                )

            def s3_ae(i, ht):
                s = S[i]
                if "A" not in s:
                    s["A"] = wk.tile(
                        [128, 4 * D], BF16, tag="A", name="A_sb", bufs=3
                    )
                    s["E"] = wk.tile(
                        [128, 4 * D], FP8, tag="E", name="E_sb", bufs=3
                    )
                A_sb, E_sb = s["A"], s["E"]
                sl = slice(ht * 512, (ht + 1) * 512)
                nc.scalar.activation(A_sb[:, sl], s[f"pa{ht}"][:], AF.Tanh)
                nc.scalar.activation(
                    E_sb[:, sl], A_sb[:, sl], AF.Exp, scale=va_sb[:, ht : ht + 1]
                )

            def s4(i):
                """att product (Pool), denominators (PE) + recip, zb."""
                s = S[i]
                E_sb, hT = s["E"], s["hT"]
                # att[k, ht*512 + j*128 + b] = E * hT (hT blocks j-major)
                s["att"] = att = wk.tile(
                    [128, 4 * D], FP8, tag="att", name="att", bufs=3
                )
                att5 = att.rearrange("k (t j b) -> k t j b", t=2, j=4)
                E5 = E_sb.rearrange("k (t j b) -> k t j b", t=2, j=4)
                hT4 = hT.rearrange("k (j kt b) -> k j kt b", j=4, kt=2)
                for ht in range(2):
                    nc.gpsimd.tensor_mul(
                        att5[:, ht], E5[:, ht], hT4[:, :, ht, :]
                    )
                pd = pdp.tile([128, 4], F32, tag="pd")
                for j in range(4):
                    for ht in range(2):
                        nc.tensor.matmul(
                            pd[:, j : j + 1],
                            E_sb[:, ht * 512 + j * 128 : ht * 512 + (j + 1) * 128],
                            ones_c[:],
                            start=(ht == 0),
                            stop=(ht == 1),
                        )
                s["r"] = r_sb = wk.tile(
                    [128, 4], F32, tag="r", name="r_sb", bufs=3
                )
                nc.vector.reciprocal(r_sb[:], pd[:])
                s["zb"] = zb = wk.tile(
                    [128, 4 * D], BF16, tag="zb", name="zb", bufs=5
                )
                nc.vector.tensor_scalar(
                    zb[:], s["tz"][:], 0.5, 0.5, op0=ALU.mult, op1=ALU.add
                )

            def s5_mm(i, jp):
                """Candidate matmuls for j-pair jp."""
                s = S[i]
                xT, att = s["xT"], s["att"]
                att4 = att.rearrange("k (t j b) -> k t j b", t=2, j=4)
                pw = pwp.tile([128, 2 * D], F32, tag="pw")
                s[f"pw{jp}"] = pw
                for jj in range(2):
                    j = jp * 2 + jj
                    for kt in range(2):
                        nc.tensor.matmul(
                            pw[:, jj * D : (jj + 1) * D],
                            xT[:, kt * 512 + j * 128 : kt * 512 + (j + 1) * 128],
                            w_sb["wht"][:, kt * D : (kt + 1) * D],
                            start=(kt == 0),
                            stop=(not use_bias and kt == 1),
                        )
                    if use_bias:
                        nc.tensor.matmul(
                            pw[:, jj * D : (jj + 1) * D],
                            ones_r[:],
                            bh_sb[:],
                            start=False,
                            stop=True,
                        )
                pu = pup.tile([128, 2 * D], F32, tag="pu")
                s[f"pu{jp}"] = pu
                for jj in range(2):
                    j = jp * 2 + jj
                    # attended @ U_h^T: one fp8 DoubleRow matmul (K=256)
                    nc.tensor.matmul(
                        pu[:, jj * D : (jj + 1) * D],
                        att4[:, :, j, :],
                        uht3[:],
                        start=True,
                        stop=True,
                        perf_mode=DR,
                    )

            def s5_fix(i, jp):
                """sW copy (ACT) + stt (DVE) for j-pair jp."""
                s = S[i]
                if "Sh" not in s:
                    s["Sh"] = wk.tile(
                        [128, 4 * D], F32, tag="Sh", name="Sh", bufs=3
                    )
                    s["sW"] = wk.tile(
                        [128, 4 * D], BF16, tag="sW", name="sW", bufs=3
                    )
                Sh, sW = s["Sh"], s["sW"]
                pw, pu = s[f"pw{jp}"], s[f"pu{jp}"]
                nc.scalar.activation(
                    sW[:, jp * 512 : (jp + 1) * 512], pw[:], AF.Copy
                )
                for jj in range(2):
                    j = jp * 2 + jj
                    nc.vector.scalar_tensor_tensor(
                        Sh[:, j * D : (j + 1) * D],
                        pu[:, jj * D : (jj + 1) * D],
                        s["r"][:, j : j + 1],
                        sW[:, j * D : (j + 1) * D],
                        op0=ALU.mult,
                        op1=ALU.add,
                    )

            def s6(i):
                s = S[i]
                s["htl"] = htl = wk.tile(
                    [128, 4 * D], BF16, tag="htl", name="htl", bufs=3
                )
                nc.scalar.activation(htl[:], s["Sh"][:], AF.Tanh)

            def s7(i):
                """Blend: h_t = h + zb*(h~ - h)."""
                s = S[i]
                s_bl = wk.tile(
                    [128, 4 * D], BF16, tag="s_bl", name="s_bl", bufs=3
                )
                nc.vector.tensor_sub(s_bl[:], s["htl"][:], s["hb"][:])
                m_bl = wk.tile(
                    [128, 4 * D], BF16, tag="m_bl", name="m_bl", bufs=3
                )
                nc.vector.tensor_mul(m_bl[:], s["zb"][:], s_bl[:])
                s["ot"] = ot = io.tile(
                    [128, 4 * D], F32, tag="ot", name="ot", bufs=4
                )
                nc.vector.tensor_add(ot[:], m_bl[:], s["hn"][:])

            def s8(i):
                s = S[i]
                b0 = i * ST
                nc.sync.dma_start(
                    out=out_d.ap()[b0 : b0 + ST, :].rearrange(
                        "(j p) k -> p j k", p=128
                    ),
                    in_=s["ot"].rearrange("p (j k) -> p j k", j=4),
                )
                S[i] = {}

            def alive(j):
                return 0 <= j < N_ST

            for _rep in range(KREP):
                if _rep == 0:
                    # tile 0's loads go ahead of the weights on the serial
                    # DMA queue; weights aren't needed until iteration 2.
                    s0(0)
                    load_weights()
                for k in range(N_ST + 7):
                    if k == 0 and _rep == 0:
                        continue  # s0(0) already emitted before the weights
                    # ACT leads with tz (unblocks PE z via the pz ring).
                    if alive(k - 3):
                        s3_tz(k - 3, 0)
                        s3_tz(k - 3, 1)
                    # PE block, interleaved to bridge ring WARs:
                    if alive(k - 5):
                        s5_mm(k - 5, 0)
                    if alive(k - 1):
                        s1(k - 1)
                    if alive(k - 2):
                        s2_zmm(k - 2, 0)
                    if alive(k - 5):
                        s5_fix(k - 5, 0)
                        s5_mm(k - 5, 1)
                    if alive(k - 2):
                        s2_zmm(k - 2, 1)
                    if alive(k - 5):
                        s5_fix(k - 5, 1)
                    if alive(k - 4):
                        s4(k - 4)
                    if alive(k - 2):
                        s2_amm(k - 2, 0)
                    if alive(k - 3):
                        s3_ae(k - 3, 0)
                    if alive(k - 2):
                        s2_amm(k - 2, 1)
                    if alive(k - 3):
                        s3_ae(k - 3, 1)
                    if alive(k - 6):
                        s6(k - 6)
                    if alive(k):
                        s0(k)
                    if alive(k - 6):
                        s7(k - 6)
                    if alive(k - 7):
                        s8(k - 7)

    nc.compile()
    return nc


LAST_RESULTS = None


def kernel(x, h_prev, W_z, U_z, b_z, W_a, U_a, v_a, W_h, U_h, b_h):
    global LAST_RESULTS
    use_bias = bool(np.any(np.asarray(b_z)) or np.any(np.asarray(b_h)))
    key = ("nc", use_bias)
    if key not in _CACHE:
        _CACHE[key] = build_nc(use_bias)
    nc = _CACHE[key]

    bf = ml_dtypes.bfloat16
    x = np.ascontiguousarray(np.asarray(x, dtype=np.float32))
    h_prev = np.ascontiguousarray(np.asarray(h_prev, dtype=np.float32))
    wmats = {
        "wzt": W_z,
        "uzt": U_z,
        "wat": W_a,
        "uat": U_a,
        "wht": W_h,
        "uht": U_h,
    }
    f8 = ml_dtypes.float8_e4m3fn
    common = {
        n: np.ascontiguousarray(
            np.asarray(m, dtype=np.float32).T.astype(f8 if n == "uht" else bf)
        )
        for n, m in wmats.items()
    }
    common["bz"] = np.asarray(b_z, dtype=np.float32).reshape(1, D).astype(bf)
    common["bh"] = np.asarray(b_h, dtype=np.float32).reshape(1, D).astype(bf)
    common["va"] = np.ascontiguousarray(np.asarray(v_a, dtype=np.float32))
    common["ident"] = np.eye(128, dtype=bf)

    in_maps = []
    for c in range(N_CORES):
        m = dict(common)
        m["x"] = x[c * B_CORE : (c + 1) * B_CORE]
        m["h"] = h_prev[c * B_CORE : (c + 1) * B_CORE]
        in_maps.append(m)

    LAST_RESULTS = run_bass_kernel_spmd(nc, in_maps, core_ids=list(range(N_CORES)))
    outs = LAST_RESULTS.results
    return np.concatenate([outs[c]["out"] for c in range(N_CORES)], axis=0)


# revision 38
# speedup vs baseline: 1.0585x; 1.0241x over previous
"""ImprovedGRUCell Trainium2 kernel (8-core data-parallel over batch).

v6: 9-stage software pipeline, one super-tile (512 rows) per stage slot.

  Stage map for super-tile n (executed at iteration n+k):
    s0 @n   : DMA loads (j-half split), f32->bf16 casts (GPSIMD).
    s1 @n+1 : xT (kt-major) and hT (j-major blocks) via PE identity-matmul
              transposes, PSUM->SBUF copybacks on DVE.
    s2 @n+2 : z + attention matmuls (PE), full-size PSUM tiles.
    s3 @n+3 : tz = tanh(Sz/2), A = tanh(Sa), E = exp(va*A) fp8 (ACT).
    s4 @n+4 : att = E*hT fp8 (GPSIMD), softmax denominators (PE) + recip
              (DVE), zb = 0.5*tz+0.5 (DVE).
    s5 @n+5 : candidate pw (bf16) + pu (fp8 DoubleRow, K=256/shot) matmuls,
              sW = Copy(pw) on ACT (stt may read only one PSUM operand),
              Sh = sW + r*pu stt on DVE.
    s6 @n+6 : htl = tanh(Sh) (ACT).
    s7 @n+7 : blend h_t = h + zb*(htl - h) (DVE).
    s8 @n+8 : store.

  Every engine's per-iteration inputs are >= 1 iteration old (or produced
  early in the same iteration by design), so per-engine programs run nearly
  stall-free; PE work is emitted interleaved (cand / xpose / z / attn) to
  bridge the PSUM-ring WAR waits and keep the systolic array dense and at
  full p-state (keeping both transposes on PE beat a DMA-xbar variant by
  ~30% end-to-end in the timeline model: the xbar's latency + DMA-queue
  contention starved PE and reset its p-state).

  PSUM (8 banks): pz[128,1024] (2) + pa[128,1024] (2) + pst transpose
  staging [128,1024]bf16 (1) + pd (1) + pw[128,512] (1) + pu[128,512] (1).
  ACT's emission order leads with tz and A so their ring reads release the
  pz/pa banks before the next tile's PE matmuls need them; single
  full-tile tanh instructions minimize ACT's per-op overhead (ACT is the
  steady-state pacer).

  Accuracy: bf16 everywhere except the attention softmax path (E, att,
  U_h-matmul operands in fp8e4; verified offline to keep rel-err at the
  bf16 level ~3e-3, budget 2e-2). Sigmoid avoided (table-set conflict with
  exp): z = 0.5*tanh(s/2)+0.5. ACT table set stays {Tanh, Exp, Copy}.
"""

import os
import sys

sys.path.insert(0, "/opt/trn_rl_repo")

import ml_dtypes
import numpy as np

import concourse.bass as bass
import concourse.mybir as mybir
from concourse import bacc, tile
from concourse.bass_utils import run_bass_kernel_spmd

B_TOTAL = 65536
N_CORES = 8
B_CORE = B_TOTAL // N_CORES  # 8192
D = 256
ST = 512  # batch rows per super-tile
N_ST = B_CORE // ST

F32 = mybir.dt.float32
BF16 = mybir.dt.bfloat16
FP8 = mybir.dt.float8e4
AF = mybir.ActivationFunctionType
ALU = mybir.AluOpType
DR = mybir.MatmulPerfMode.DoubleRow

_CACHE = {}

WNAMES = ("wzt", "uzt", "wat", "uat", "wht", "uht")


def build_nc(use_bias=False):
    nc = bacc.Bacc(
        "TRN2",
        target_bir_lowering=False,
        debug=False,
        enable_asserts=False,
        num_devices=N_CORES,
    )

    x_d = nc.dram_tensor("x", [B_CORE, D], F32, kind="ExternalInput")
    h_d = nc.dram_tensor("h", [B_CORE, D], F32, kind="ExternalInput")
    w_d = {
        n: nc.dram_tensor(
            n, [D, D], FP8 if n == "uht" else BF16, kind="ExternalInput"
        )
        for n in WNAMES
    }
    bz_d = nc.dram_tensor("bz", [1, D], BF16, kind="ExternalInput")
    bh_d = nc.dram_tensor("bh", [1, D], BF16, kind="ExternalInput")
    va_d = nc.dram_tensor("va", [D], F32, kind="ExternalInput")
    id_d = nc.dram_tensor("ident", [128, 128], BF16, kind="ExternalInput")
    out_d = nc.dram_tensor("out", [B_CORE, D], F32, kind="ExternalOutput")

    with tile.TileContext(nc) as tc:
        with (
            tc.tile_pool(name="wpool", bufs=1) as wp,
            tc.tile_pool(name="io", bufs=3) as io,
            tc.tile_pool(name="wk", bufs=3) as wk,
            tc.tile_pool(name="pz", bufs=1, space="PSUM") as pzp,
            tc.tile_pool(name="pa", bufs=1, space="PSUM") as pap,
            tc.tile_pool(name="pst", bufs=1, space="PSUM") as pst,
            tc.tile_pool(name="pd", bufs=1, space="PSUM") as pdp,
            tc.tile_pool(name="pw", bufs=1, space="PSUM") as pwp,
            tc.tile_pool(name="pu", bufs=1, space="PSUM") as pup,
        ):
            # ---- persistent weights -------------------------------------
            # w_sb[n]: [128 part = in-feature%128, (kt, hid)]; uht in fp8.
            # Tiles are allocated here but the weight DMAs are EMITTED after
            # tile 0's x/h loads (load_weights below): the DMA queue is
            # serial and the first super-tile's data is needed first, while
            # weights aren't consumed until the first matmuls two
            # iterations later.
            w_sb = {}
            for n in WNAMES:
                dt = FP8 if n == "uht" else BF16
                w_sb[n] = wp.tile([128, 2 * D], dt, tag=n, name=n)
            bz_sb = wp.tile([1, D], BF16, tag="bz")
            bh_sb = wp.tile([1, D], BF16, tag="bh")
            va_sb = wp.tile([128, 2], F32, tag="va")
            ones_r = wp.tile([1, 128], BF16, tag="ones_r")  # K=1 lhsT for bias
            nc.vector.memset(ones_r[:], 1.0)
            ones_c = wp.tile([128, 1], BF16, tag="ones_c")  # rhs for denom
            nc.vector.memset(ones_c[:], 1.0)
            ident = wp.tile([128, 128], BF16, tag="ident")

            def load_weights():
                nc.sync.dma_start(out=ident[:], in_=id_d.ap())
                for n in WNAMES:
                    nc.sync.dma_start(
                        out=w_sb[n].rearrange("p (kt h) -> p kt h", kt=2),
                        in_=w_d[n].ap().rearrange("(kt p) h -> p kt h", p=128),
                    )
                nc.sync.dma_start(out=bz_sb[:], in_=bz_d.ap())
                nc.sync.dma_start(out=bh_sb[:], in_=bh_d.ap())
                nc.sync.dma_start(
                    out=va_sb[:], in_=va_d.ap().rearrange("(t p) -> p t", p=128)
                )

            uht3 = w_sb["uht"].rearrange("p (t h) -> p t h", t=2)
            # KREP>1 repeats the whole batch loop for wall-clock benching.
            KREP = int(os.environ.get("KREP", "1"))
            S = [dict() for _ in range(N_ST)]

            def s0(i):
                """Loads + casts + h xbar transpose."""
                s = S[i]
                b0 = i * ST
                s["xn"] = xn = io.tile(
                    [128, 4 * D], F32, tag="xn", name="xn", bufs=4
                )
                s["hn"] = hn = io.tile(
                    [128, 4 * D], F32, tag="hn", name="hn", bufs=9
                )
                s["xb"] = xb = wk.tile(
                    [128, 4 * D], BF16, tag="xb", name="xb", bufs=4
                )
                s["hb"] = hb = wk.tile(
                    [128, 4 * D], BF16, tag="hb", name="hb", bufs=9
                )
                for src, dst, cst in ((x_d, xn, xb), (h_d, hn, hb)):
                    nc.sync.dma_start(
                        out=dst.rearrange("p (j k) -> p j k", j=4),
                        in_=src.ap()[b0 : b0 + ST, :].rearrange(
                            "(j p) k -> p j k", p=128
                        ),
                    )
                    nc.gpsimd.tensor_copy(cst[:], dst[:])

            def s1(i):
                """xT via PE transpose (kt-major: xT[kk, kt*512+j*128+b]);
                hT likewise on PE but stored j-major (hT[kk,(j*2+kt)*128+b])
                so downstream slicing matches the xbar layout."""
                s = S[i]
                xb, hb = s["xb"], s["hb"]
                s["xT"] = xT = wk.tile(
                    [128, 4 * D], BF16, tag="xT", name="xT", bufs=5
                )
                s["hT"] = hT = wk.tile(
                    [128, 4 * D], BF16, tag="hT", name="hT", bufs=5
                )
                ptx = pst.tile([128, 4 * D], BF16, tag="pst")
                ptx3 = ptx.rearrange("k (kt j b) -> k (kt j) b", kt=2, j=4)
                for kt in range(2):
                    for j in range(4):
                        nc.tensor.transpose(
                            ptx3[:, kt * 4 + j],
                            xb[:, j * D + kt * 128 : j * D + (kt + 1) * 128],
                            ident[:],
                        )
                nc.vector.tensor_copy(xT[:], ptx[:])
                pth = pst.tile([128, 4 * D], BF16, tag="pst")
                pth3 = pth.rearrange("k (j kt b) -> k (j kt) b", kt=2, j=4)
                for kt in range(2):
                    for j in range(4):
                        nc.tensor.transpose(
                            pth3[:, j * 2 + kt],
                            hb[:, j * D + kt * 128 : j * D + (kt + 1) * 128],
                            ident[:],
                        )
                nc.vector.tensor_copy(hT[:], pth[:])

            def s2_zmm(i, jp):
                """z-branch matmuls for j-pair jp (full-size tile)."""
                s = S[i]
                xT, hT = s["xT"], s["hT"]
                if jp == 0:
                    s["pz"] = pzp.tile([128, 4 * D], F32, tag="pz", name="pz")
                pz = s["pz"]
                for jj in range(2):
                    j = jp * 2 + jj
                    sl = slice(j * D, (j + 1) * D)
                    for kt in range(2):
                        nc.tensor.matmul(
                            pz[:, sl],
                            xT[:, kt * 512 + j * 128 : kt * 512 + (j + 1) * 128],
                            w_sb["wzt"][:, kt * D : (kt + 1) * D],
                            start=(kt == 0),
                            stop=False,
                        )
                    for kt in range(2):
                        nc.tensor.matmul(
                            pz[:, sl],
                            hT[:, (j * 2 + kt) * 128 : (j * 2 + kt + 1) * 128],
                            w_sb["uzt"][:, kt * D : (kt + 1) * D],
                            start=False,
                            stop=(not use_bias and kt == 1),
                        )
                    if use_bias:
                        nc.tensor.matmul(
                            pz[:, sl], ones_r[:], bz_sb[:], start=False, stop=True
                        )

            def s2_amm(i, ht):
                """Attention matmuls for hidden-half ht (full-size tile)."""
                s = S[i]
                xT, hT = s["xT"], s["hT"]
                if ht == 0:
                    s["pa"] = pap.tile([128, 4 * D], F32, tag="pa", name="pa")
                pa = s["pa"][:, ht * 512 : (ht + 1) * 512]
                hT4 = hT.rearrange("k (j kt b) -> k j kt b", j=4, kt=2)
                for kt in range(2):
                    nc.tensor.matmul(
                        pa,
                        w_sb["wat"][
                            :, kt * D + ht * 128 : kt * D + ht * 128 + 128
                        ],
                        xT[:, kt * 512 : (kt + 1) * 512],
                        start=(kt == 0),
                        stop=False,
                    )
                for kt in range(2):
                    nc.tensor.matmul(
                        pa,
                        w_sb["uat"][
                            :, kt * D + ht * 128 : kt * D + ht * 128 + 128
                        ],
                        hT4[:, :, kt, :],
                        start=False,
                        stop=(kt == 1),
                    )

            def s3_tz(i):
                s = S[i]
                s["tz"] = tz = wk.tile(
                    [128, 4 * D], BF16, tag="tz", name="tz", bufs=3
                )
                nc.scalar.activation(tz[:], s["pz"][:], AF.Tanh, scale=0.5)

            def s3_a(i):
                s = S[i]
                s["A"] = A_sb = wk.tile(
                    [128, 4 * D], BF16, tag="A", name="A_sb", bufs=3
                )
                nc.scalar.activation(A_sb[:], s["pa"][:], AF.Tanh)

            def s3_e(i):
                s = S[i]
                A_sb = s["A"]
                s["E"] = E_sb = wk.tile(
                    [128, 4 * D], FP8, tag="E", name="E_sb", bufs=3
                )
                for ht in range(2):
                    sl = slice(ht * 512, (ht + 1) * 512)
                    nc.scalar.activation(
                        E_sb[:, sl],
                        A_sb[:, sl],
                        AF.Exp,
                        scale=va_sb[:, ht : ht + 1],
                    )

            def s4(i):
                """att product (Pool), denominators (PE) + recip, zb."""
                s = S[i]
                E_sb, hT = s["E"], s["hT"]
                # att[k, ht*512 + j*128 + b] = E * hT (hT blocks j-major)
                s["att"] = att = wk.tile(
                    [128, 4 * D], FP8, tag="att", name="att", bufs=3
                )
                att5 = att.rearrange("k (t j b) -> k t j b", t=2, j=4)
                E5 = E_sb.rearrange("k (t j b) -> k t j b", t=2, j=4)
                hT4 = hT.rearrange("k (j kt b) -> k j kt b", j=4, kt=2)
                for ht in range(2):
                    nc.gpsimd.tensor_mul(
                        att5[:, ht], E5[:, ht], hT4[:, :, ht, :]
                    )
                pd = pdp.tile([128, 4], F32, tag="pd")
                for j in range(4):
                    for ht in range(2):
                        nc.tensor.matmul(
                            pd[:, j : j + 1],
                            E_sb[:, ht * 512 + j * 128 : ht * 512 + (j + 1) * 128],
                            ones_c[:],
                            start=(ht == 0),
                            stop=(ht == 1),
                        )
                s["r"] = r_sb = wk.tile(
                    [128, 4], F32, tag="r", name="r_sb", bufs=3
                )
                nc.vector.reciprocal(r_sb[:], pd[:])
                s["zb"] = zb = wk.tile(
                    [128, 4 * D], BF16, tag="zb", name="zb", bufs=5
                )
                nc.vector.tensor_scalar(
                    zb[:], s["tz"][:], 0.5, 0.5, op0=ALU.mult, op1=ALU.add
                )

            def s5_mm(i, jp):
                """Candidate matmuls for j-pair jp."""
                s = S[i]
                xT, att = s["xT"], s["att"]
                att4 = att.rearrange("k (t j b) -> k t j b", t=2, j=4)
                pw = pwp.tile([128, 2 * D], F32, tag="pw")
                s[f"pw{jp}"] = pw
                for jj in range(2):
                    j = jp * 2 + jj
                    for kt in range(2):
                        nc.tensor.matmul(
                            pw[:, jj * D : (jj + 1) * D],
                            xT[:, kt * 512 + j * 128 : kt * 512 + (j + 1) * 128],
                            w_sb["wht"][:, kt * D : (kt + 1) * D],
                            start=(kt == 0),
                            stop=(not use_bias and kt == 1),
                        )
                    if use_bias:
                        nc.tensor.matmul(
                            pw[:, jj * D : (jj + 1) * D],
                            ones_r[:],
                            bh_sb[:],
                            start=False,
                            stop=True,
                        )
                pu = pup.tile([128, 2 * D], F32, tag="pu")
                s[f"pu{jp}"] = pu
                for jj in range(2):
                    j = jp * 2 + jj
                    # attended @ U_h^T: one fp8 DoubleRow matmul (K=256)
                    nc.tensor.matmul(
                        pu[:, jj * D : (jj + 1) * D],
                        att4[:, :, j, :],
                        uht3[:],
                        start=True,
                        stop=True,
                        perf_mode=DR,
                    )

            def s5_fix(i, jp):
                """sW copy (ACT) + stt (DVE) for j-pair jp."""
                s = S[i]
                if "Sh" not in s:
                    s["Sh"] = wk.tile(
                        [128, 4 * D], F32, tag="Sh", name="Sh", bufs=3
                    )
                    s["sW"] = wk.tile(
                        [128, 4 * D], BF16, tag="sW", name="sW", bufs=3
                    )
                Sh, sW = s["Sh"], s["sW"]
                pw, pu = s[f"pw{jp}"], s[f"pu{jp}"]
                nc.scalar.activation(
                    sW[:, jp * 512 : (jp + 1) * 512], pw[:], AF.Copy
                )
                for jj in range(2):
                    j = jp * 2 + jj
                    nc.vector.scalar_tensor_tensor(
                        Sh[:, j * D : (j + 1) * D],
                        pu[:, jj * D : (jj + 1) * D],
                        s["r"][:, j : j + 1],
                        sW[:, j * D : (j + 1) * D],
                        op0=ALU.mult,
                        op1=ALU.add,
                    )

            def s6(i):
                s = S[i]
                s["htl"] = htl = wk.tile(
                    [128, 4 * D], BF16, tag="htl", name="htl", bufs=3
                )
                nc.scalar.activation(htl[:], s["Sh"][:], AF.Tanh)

            def s7(i):
                """Blend: h_t = h + zb*(h~ - h)."""
                s = S[i]
                s_bl = wk.tile(
                    [128, 4 * D], BF16, tag="s_bl", name="s_bl", bufs=3
                )
                nc.vector.tensor_sub(s_bl[:], s["htl"][:], s["hb"][:])
                m_bl = wk.tile(
                    [128, 4 * D], BF16, tag="m_bl", name="m_bl", bufs=3
                )
                nc.vector.tensor_mul(m_bl[:], s["zb"][:], s_bl[:])
                s["ot"] = ot = io.tile(
                    [128, 4 * D], F32, tag="ot", name="ot", bufs=4
                )
                nc.vector.tensor_add(ot[:], m_bl[:], s["hn"][:])

            def s8(i):
                s = S[i]
                b0 = i * ST
                nc.sync.dma_start(
                    out=out_d.ap()[b0 : b0 + ST, :].rearrange(
                        "(j p) k -> p j k", p=128
                    ),
                    in_=s["ot"].rearrange("p (j k) -> p j k", j=4),
                )
                S[i] = {}

            def alive(j):
                return 0 <= j < N_ST

            for _rep in range(KREP):
                if _rep == 0:
                    # tile 0's loads go ahead of the weights on the serial
                    # DMA queue; weights aren't needed until iteration 2.
                    s0(0)
                    load_weights()
                for k in range(N_ST + 7):
                    if k == 0 and _rep == 0:
                        continue  # s0(0) already emitted before the weights
                    # ACT leads with tz and A (their reads release the pz/pa
                    # rings before PE's matmuls of the next tile need them).
                    if alive(k - 3):
                        s3_tz(k - 3)
                        s3_a(k - 3)
                    # PE block, interleaved to bridge ring WARs:
                    if alive(k - 5):
                        s5_mm(k - 5, 0)
                    if alive(k - 1):
                        s1(k - 1)
                    if alive(k - 2):
                        s2_zmm(k - 2, 0)
                    if alive(k - 5):
                        s5_fix(k - 5, 0)
                        s5_mm(k - 5, 1)
                    if alive(k - 2):
                        s2_zmm(k - 2, 1)
                    if alive(k - 5):
                        s5_fix(k - 5, 1)
                    if alive(k - 4):
                        s4(k - 4)
                    if alive(k - 2):
                        s2_amm(k - 2, 0)
                        s2_amm(k - 2, 1)
                    if alive(k - 3):
                        s3_e(k - 3)
                    if alive(k - 6):
                        s6(k - 6)
                    if alive(k):
                        s0(k)
                    if alive(k - 6):
                        s7(k - 6)
                    if alive(k - 7):
                        s8(k - 7)

    nc.compile()
    return nc


LAST_RESULTS = None


def kernel(x, h_prev, W_z, U_z, b_z, W_a, U_a, v_a, W_h, U_h, b_h):
    global LAST_RESULTS
    use_bias = bool(np.any(np.asarray(b_z)) or np.any(np.asarray(b_h)))
    key = ("nc", use_bias)
    if key not in _CACHE:
        _CACHE[key] = build_nc(use_bias)
    nc = _CACHE[key]

    bf = ml_dtypes.bfloat16
    x = np.ascontiguousarray(np.asarray(x, dtype=np.float32))
    h_prev = np.ascontiguousarray(np.asarray(h_prev, dtype=np.float32))
    wmats = {
        "wzt": W_z,
        "uzt": U_z,
        "wat": W_a,
        "uat": U_a,
        "wht": W_h,
        "uht": U_h,
    }
    f8 = ml_dtypes.float8_e4m3fn
    common = {
        n: np.ascontiguousarray(
            np.asarray(m, dtype=np.float32).T.astype(f8 if n == "uht" else bf)
        )
        for n, m in wmats.items()
    }
    common["bz"] = np.asarray(b_z, dtype=np.float32).reshape(1, D).astype(bf)
    common["bh"] = np.asarray(b_h, dtype=np.float32).reshape(1, D).astype(bf)
    common["va"] = np.ascontiguousarray(np.asarray(v_a, dtype=np.float32))
    common["ident"] = np.eye(128, dtype=bf)

    in_maps = []
    for c in range(N_CORES):
        m = dict(common)
        m["x"] = x[c * B_CORE : (c + 1) * B_CORE]
        m["h"] = h_prev[c * B_CORE : (c + 1) * B_CORE]
        in_maps.append(m)

    LAST_RESULTS = run_bass_kernel_spmd(nc, in_maps, core_ids=list(range(N_CORES)))
    outs = LAST_RESULTS.results
    return np.concatenate([outs[c]["out"] for c in range(N_CORES)], axis=0)
